# revision 30
# baseline (speedup 1.0000x reference)
"""Bi-directional cross-attention kernel for Trainium2 (8 NeuronCores).

Sharding: data-parallel over batch B=8 -> one batch element per core (SPMD,
no collectives). Each core computes the full bidirectional cross-attention
for its batch element.

Per-core layout strategy (C=256 channels, S=1024 tokens, 8 heads x 64 dim):
  - K1col/K2col: [512, 1024]  (head-major rows on partitions, tokens free)
  - V1aug/V2aug: row layout [1024 tokens, 8*(64+1)] with a ones-column per
    head; the ones-column makes the attention matmul emit the softmax
    denominator as PSUM row 64 for free.
  - scores are never max-shifted (values ~N(0, 0.01) after the 1/8 scale,
    exp is safe); exp(0.125*s) fused into the PSUM->SBUF copy on ScalarE.
  - E^T produced by xbar DMA transposes of the E tiles (issued right after
    each exp, all on the SP HWDGE ring -- mixing rings corrupts data on this
    runtime) -- keeps the second softmax orientation OFF the ScalarE
    critical path entirely; the transfers hide under compute.
  - normalization: the PSUM accumulator is staged to SBUF with one copy so
    its bank frees immediately; 1/denom via the single-instruction
    custom-DVE reciprocal_approx_fast (~51 ULP, plenty for a softmax
    denominator; it must read a base-0 full tile -- partition-offset APs
    are mishandled on HW), broadcast across partitions via a K=1 matmul
    against an all-ones row, then one VectorE multiply (in1 straight from
    PSUM).
  - output projection packs head PAIRS (K=128 contraction instead of two
    K=64 chains), halving its PE time.
  - output projection biases (including the folded V-bias contribution) are
    applied per-partition during the final PSUM->SBUF copy on VectorE.
  - iters>1 wraps the body in a tc.For_i hardware loop (body emitted once,
    replayed on device) -- used by test.py to measure marginal per-iteration
    HW time without NEFF-size-dependent host overhead.
"""

import os
import sys

for _p in ("/opt/trn_rl_repo", os.path.expanduser("~/.axon_site/_ro/trn_rl_repo")):
    if os.path.isdir(_p) and _p not in sys.path:
        sys.path.insert(0, _p)

import numpy as np
import ml_dtypes

import concourse.bass as bass
import concourse.tile as tile
import concourse.mybir as mybir
from concourse import bacc

BF16 = mybir.dt.bfloat16
F32 = mybir.dt.float32
AF = mybir.ActivationFunctionType
ALU = mybir.AluOpType

B = 8
C = 256          # channels per image
S = 1024         # tokens per image (32*32)
NH = 8           # heads
HD = 64          # head dim
J = NH * HD      # 512
P = 128
NCC = C // P     # 2 channel chunks
NQ = S // P      # 8 token chunks
NKB = S // 512   # 2 psum banks across tokens
HB = HD + 1      # head block width in V-aug (64 d + ones col)
NP = NH // 2     # head pairs


def _emit(nc: bass.Bass, iters: int = 1,
          transpose_mode: str = "xbar_sp",
          ones_engine: str = "pool",
          norm_mode: str = "recip",
          unroll: int = 1,
          ablate: frozenset = frozenset()) -> None:
    x1 = nc.declare_dram_parameter("x1", [C, S], BF16, isOutput=False)
    x2 = nc.declare_dram_parameter("x2", [C, S], BF16, isOutput=False)
    wk1 = nc.declare_dram_parameter("wk1", [C, J], BF16, isOutput=False)
    wk2 = nc.declare_dram_parameter("wk2", [C, J], BF16, isOutput=False)
    wv1 = nc.declare_dram_parameter("wv1", [C, J], BF16, isOutput=False)
    wv2 = nc.declare_dram_parameter("wv2", [C, J], BF16, isOutput=False)
    wo1 = nc.declare_dram_parameter("wo1", [J, C], BF16, isOutput=False)
    wo2 = nc.declare_dram_parameter("wo2", [J, C], BF16, isOutput=False)
    bk1 = nc.declare_dram_parameter("bk1", [P, J // P], F32, isOutput=False)
    bk2 = nc.declare_dram_parameter("bk2", [P, J // P], F32, isOutput=False)
    bo1 = nc.declare_dram_parameter("bo1", [P, NCC], F32, isOutput=False)
    bo2 = nc.declare_dram_parameter("bo2", [P, NCC], F32, isOutput=False)
    y1 = nc.declare_dram_parameter("y1", [C, S], F32, isOutput=True)
    y2 = nc.declare_dram_parameter("y2", [C, S], F32, isOutput=True)

    with tile.TileContext(nc) as tc:
        with (
            tc.tile_pool(name="const", bufs=1) as cp,
            tc.tile_pool(name="work", bufs=2) as wp,
            tc.tile_pool(name="norm", bufs=4) as np_,
            tc.tile_pool(name="psA", bufs=2, space="PSUM") as psA,
            tc.tile_pool(name="psB", bufs=2, space="PSUM") as psB,
            tc.tile_pool(name="psC", bufs=2, space="PSUM") as psC,
        ):
            # all-ones row at partition 64, used to broadcast the recip row
            # across partitions via a K=1 matmul.
            ones_t = cp.tile([HD + 1, HD], BF16, tag="ones_t", name="ones_t")
            nc.vector.memset(ones_t[:], 1.0)

            def _body_loop():
                if iters == 1:
                    yield 0
                else:
                    # hardware loop: one body copy in the NEFF, replayed
                    # iters times back-to-back on device. unroll>1 emits the
                    # body several times per loop trip (fewer back-edge
                    # all-engine barriers).
                    assert iters % unroll == 0
                    with tc.For_i(0, iters // unroll, 1):
                        for _u in range(unroll):
                            yield _u

            def load(dram, shape, dtype, tag):
                # SWDGE (Pool-issued) ring: keeps the SP HWDGE ring free
                # for the E^T xbar transposes
                t = cp.tile(shape, dtype, tag=tag, name=tag)
                nc.gpsimd.dma_start(out=t[:], in_=dram[:])
                return t

            # weights and biases are loop-invariant: loaded once, resident in
            # SBUF across iterations (weight-stationary steady state)
            wk1_sb = [load(wk1[cc * P:(cc + 1) * P, :], [P, J], BF16, f"wk1_{cc}")
                      for cc in range(NCC)]
            wk2_sb = [load(wk2[cc * P:(cc + 1) * P, :], [P, J], BF16, f"wk2_{cc}")
                      for cc in range(NCC)]
            wv1_sb = [load(wv1[cc * P:(cc + 1) * P, :], [P, J], BF16, f"wv1_{cc}")
                      for cc in range(NCC)]
            wv2_sb = [load(wv2[cc * P:(cc + 1) * P, :], [P, J], BF16, f"wv2_{cc}")
                      for cc in range(NCC)]
            # o-projection weights: one [128, C] tile per head PAIR so the
            # out-projection contracts K=128 per matmul.
            wo1_sb = [load(wo1[hp * P:(hp + 1) * P, :], [P, C], BF16, f"wo1_{hp}")
                      for hp in range(NP)]
            wo2_sb = [load(wo2[hp * P:(hp + 1) * P, :], [P, C], BF16, f"wo2_{hp}")
                      for hp in range(NP)]
            bk1_sb = load(bk1, [P, J // P], F32, "bk1")
            bk2_sb = load(bk2, [P, J // P], F32, "bk2")
            bo1_sb = load(bo1, [P, NCC], F32, "bo1")
            bo2_sb = load(bo2, [P, NCC], F32, "bo2")

            for _it in _body_loop():
                # ---- load activations -------------------------------------------
                def load2(dram, shape, dtype, tag):
                    # bufs=2 so the next loop copy's loads overlap this
                    # copy's compute tail
                    t = cp.tile(shape, dtype, tag=tag, name=tag, bufs=2)
                    nc.gpsimd.dma_start(out=t[:], in_=dram[:])
                    return t

                x1_sb = [load2(x1[cc * P:(cc + 1) * P, :], [P, S], BF16, f"x1_{cc}")
                         for cc in range(NCC)]
                x2_sb = [load2(x2[cc * P:(cc + 1) * P, :], [P, S], BF16, f"x2_{cc}")
                         for cc in range(NCC)]

                # ---- K projections: Kcol[j, s] = sum_c wk[c, j] * x[c, s] + bk ---
                def k_proj(x_sb, w_sb, b_sb, tag):
                    out = []
                    for m in range(J // P):
                        ps = psA.tile([P, S], F32, tag="pe", name="pe")
                        for nb in range(NKB):
                            for cc in range(NCC):
                                nc.tensor.matmul(
                                    ps[:, nb * 512:(nb + 1) * 512],
                                    lhsT=w_sb[cc][:, m * P:(m + 1) * P],
                                    rhs=x_sb[cc][:, nb * 512:(nb + 1) * 512],
                                    start=(cc == 0), stop=(cc == NCC - 1),
                                )
                        k_sb = cp.tile([P, S], BF16, tag=f"{tag}_{m}", name=f"{tag}_{m}",
                                       bufs=2)
                        nc.vector.tensor_scalar(k_sb[:], ps[:],
                                                b_sb[:, m:m + 1], None, ALU.add)
                        out.append(k_sb)
                    return out

                K1_sb = k_proj(x1_sb, wk1_sb, bk1_sb, "k1")
                K2_sb = k_proj(x2_sb, wk2_sb, bk2_sb, "k2")

                # ---- V projections into augmented row layout ---------------------
                # Vaug[qc] : [128 tokens, 8*(64+1)] ; per-head 64 values + ones col
                def v_proj(x_sb, w_sb, tag):
                    out = []
                    for qc in range(NQ):
                        ps = psB.tile([P, J], F32, tag="po", name="po")
                        for cc in range(NCC):
                            nc.tensor.matmul(
                                ps[:],
                                lhsT=x_sb[cc][:, qc * P:(qc + 1) * P],
                                rhs=w_sb[cc][:],
                                start=(cc == 0), stop=(cc == NCC - 1),
                            )
                        va = cp.tile([P, NH * HB], BF16, tag=f"{tag}_{qc}", name=f"{tag}_{qc}")
                        va_v = va[:].rearrange("p (h c) -> p h c", c=HB)
                        ps_v = ps[:].rearrange("p (h c) -> p h c", c=HD)
                        nc.vector.tensor_copy(va_v[:, :, 0:HD], ps_v)
                        if ones_engine == "pool":
                            nc.gpsimd.memset(va_v[:, :, HD:HB], 1.0)
                        else:
                            nc.vector.memset(va_v[:, :, HD:HB], 1.0)
                        out.append(va)
                    return out

                V1a_sb = v_proj(x1_sb, wv1_sb, "v1a")
                V2a_sb = v_proj(x2_sb, wv2_sb, "v2a")

                # output tiles per head PAIR: [128, S] (head A rows 0-63,
                # head B rows 64-127) so out_proj contracts K=128.
                O1_sb = [cp.tile([P, S], BF16, tag=f"o1_{hp}", name=f"o1_{hp}")
                         for hp in range(NP)]
                O2_sb = [cp.tile([P, S], BF16, tag=f"o2_{hp}", name=f"o2_{hp}")
                         for hp in range(NP)]

                def normalize(po, o_sb, r0, nb):
                    if "nonorm" in ablate:  # timing ablation only
                        nc.vector.tensor_copy(
                            o_sb[r0:r0 + HD, nb * 512:(nb + 1) * 512], po[0:HD, :])
                        return
                    """po: [65, 512] psum (rows 0..63 unnormalized out, row 64
                    the softmax denominator). Writes o_sb[r0:r0+64, nb*512:...].
                    One DVE copy stages po to SBUF so the PSUM slot frees
                    immediately (the accumulation chains never wait on the
                    normalize latency); then 1/denom on DVE (single custom
                    op over the full base-0 tile -- the op mishandles
                    partition-offset APs on HW), broadcast across the 64
                    partitions via a K=1 matmul, one DVE mult (in1 read
                    straight from PSUM)."""
                    u = np_.tile([HD + 1, 512], F32, tag="u", name="u", bufs=2)
                    nc.vector.tensor_copy(u[:], po[:])
                    rb = np_.tile([HD + 1, 512], BF16, tag="rb", name="rb",
                                  bufs=2)
                    if norm_mode == "recip":
                        # recip runs straight off PSUM (base-0 full tile);
                        # the bf16 u staging copy runs in parallel
                        rr = np_.tile([HD + 1, 512], F32, tag="rr", name="rr",
                                      bufs=2)
                        nc.vector.reciprocal_approx_fast(out=rr[:], in_=po[:])
                        nc.gpsimd.tensor_copy(rb[HD:HD + 1, :], rr[HD:HD + 1, :])
                    else:  # ln/exp on ScalarE (baseline scheme)
                        lt = np_.tile([HD + 1, 512], F32, tag="lt", name="lt")
                        nc.scalar.activation(lt[HD:HD + 1, :], po[HD:HD + 1, :], AF.Ln)
                        nc.scalar.activation(rb[HD:HD + 1, :], lt[HD:HD + 1, :], AF.Exp,
                                             scale=-1.0)
                    bc_ps = psC.tile([HD, 512], F32, tag="bc", name="bc_ps")
                    nc.tensor.matmul(bc_ps[:], lhsT=ones_t[HD:HD + 1, :],
                                     rhs=rb[HD:HD + 1, :], start=True, stop=True)
                    nc.vector.tensor_tensor(
                        out=o_sb[r0:r0 + HD, nb * 512:(nb + 1) * 512],
                        in0=u[0:HD, :], in1=bc_ps[:], op=ALU.mult)

                if "floor" in ablate:  # timing ablation: no attention at all
                    for hp0 in range(NP):
                        nc.vector.memset(O1_sb[hp0][:], 0.001)
                        nc.vector.memset(O2_sb[hp0][:], 0.001)
                # ---- attention, processed in head pairs -------------------------
                # the two heads of a pair occupy array row groups 0-63 and
                # 64-127 (lhsT/rhs partition base auto-derives tile_position),
                # so their K=64 score matmuls run concurrently on the PE.
                for hp in (range(0) if "floor" in ablate else range(NP)):
                    pair = (2 * hp, 2 * hp + 1)
                    e_sb = {h: [] for h in pair}
                    et = {}
                    for h in pair:
                        et[h] = wp.tile([P, NQ, S], BF16, tag=f"et{h % 2}",
                                        name="et", bufs=1)
                    for qc in range(NQ):
                        pse = {}
                        for h in pair:
                            r0 = HD * (h % 2)
                            ps = psA.tile([P, S], F32, tag="pe", name="pe")
                            for nb in range(NKB):
                                nc.tensor.matmul(
                                    ps[:, nb * 512:(nb + 1) * 512],
                                    lhsT=K1_sb[hp][r0:r0 + HD, qc * P:(qc + 1) * P],
                                    rhs=K2_sb[hp][r0:r0 + HD, nb * 512:(nb + 1) * 512],
                                    start=True, stop=True,
                                )
                            pse[h] = ps
                        for h in pair:
                            e = wp.tile([P, S], BF16, tag=f"e{qc}", name=f"e{qc}",
                                        bufs=4)
                            if "smallexp" in ablate:  # timing ablation only
                                nc.scalar.activation(e[:, 0:P], pse[h][:, 0:P],
                                                     AF.Exp, scale=0.125)
                            else:
                                nc.scalar.activation(e[:], pse[h][:], AF.Exp,
                                                     scale=0.125)
                            e_sb[h].append(e)
                            if (transpose_mode in ("xbar", "xbar_sp", "xbar_h")
                                    and "noet" not in ablate):
                                # E^T tile for this q-chunk via xbar DMA
                                # transpose; "xbar" alternates HWDGE rings
                                # (SP / ACT) per chunk, "xbar_sp" pins all to
                                # SP, "xbar_h" pins per-head (each et tile
                                # written by exactly one ring)
                                if transpose_mode == "xbar_sp":
                                    eng = nc.sync
                                elif transpose_mode == "xbar_h":
                                    eng = nc.sync if h % 2 == 0 else nc.scalar
                                else:
                                    eng = nc.sync if (qc + h) % 2 == 0 else nc.scalar
                                eng.dma_start(
                                    out=et[h][:, :, qc * P:(qc + 1) * P],
                                    in_=e[:],
                                    transpose=True,
                                )

                    if transpose_mode == "recompute":
                        # E^T = exp(0.125 * K2_h^T K1_h) computed directly
                        for h in pair:
                            r0 = HD * (h % 2)
                            for kc in range(NQ):
                                ps = psA.tile([P, S], F32, tag="pe", name="pe")
                                for nb in range(NKB):
                                    nc.tensor.matmul(
                                        ps[:, nb * 512:(nb + 1) * 512],
                                        lhsT=K2_sb[hp][r0:r0 + HD,
                                                       kc * P:(kc + 1) * P],
                                        rhs=K1_sb[hp][r0:r0 + HD,
                                                      nb * 512:(nb + 1) * 512],
                                        start=True, stop=True,
                                    )
                                nc.scalar.activation(et[h][:, kc, :], ps[:],
                                                     AF.Exp, scale=0.125)

                    # out2 chains (need only E) for both heads first, giving
                    # the xbar transposes time to land before out1 needs E^T
                    for h in pair:
                        if "noout2" in ablate:
                            break
                        r0 = HD * (h % 2)
                        # out2[d, k] = sum_q V1[q, h*64+d]*E[q, k] (+denominator)
                        for nb in range(NKB):
                            po = psB.tile([HB, 512], F32, tag="po", name="po")
                            for qc in range(NQ):
                                nc.tensor.matmul(
                                    po[:],
                                    lhsT=V1a_sb[qc][:, h * HB:(h + 1) * HB],
                                    rhs=e_sb[h][qc][:, nb * 512:(nb + 1) * 512],
                                    start=(qc == 0), stop=(qc == NQ - 1),
                                )
                            normalize(po, O2_sb[hp], r0, nb)

                    for h in pair:
                        if "noout1" in ablate:
                            break
                        r0 = HD * (h % 2)
                        # out1[d, q] = sum_k V2[k, h*64+d]*E[q, k] (+denominator)
                        for nb in range(NKB):
                            po = psB.tile([HB, 512], F32, tag="po", name="po")
                            for kc in range(NQ):
                                rhs = (e_sb[h][kc][:, nb * 512:(nb + 1) * 512]
                                       if "noet" in ablate else
                                       et[h][:, kc, nb * 512:(nb + 1) * 512])
                                nc.tensor.matmul(
                                    po[:],
                                    lhsT=V2a_sb[kc][:, h * HB:(h + 1) * HB],
                                    rhs=rhs,
                                    start=(kc == 0), stop=(kc == NQ - 1),
                                )
                            normalize(po, O1_sb[hp], r0, nb)

                # ---- output projections (head-pair K=128 contraction) -----------
                def out_proj(o_sb, wo_sb, bo_sb, y):
                    for mc in range(NCC):
                        ps = psA.tile([P, S], F32, tag="pe", name="pe")
                        for nb in range(NKB):
                            for hp in range(NP):
                                nc.tensor.matmul(
                                    ps[:, nb * 512:(nb + 1) * 512],
                                    lhsT=wo_sb[hp][:, mc * P:(mc + 1) * P],
                                    rhs=o_sb[hp][:, nb * 512:(nb + 1) * 512],
                                    start=(hp == 0), stop=(hp == NP - 1),
                                )
                        ysb = wp.tile([P, S], F32, tag="y", name="y")
                        nc.vector.tensor_scalar(ysb[:], ps[:],
                                                bo_sb[:, mc:mc + 1], None, ALU.add)
                        nc.gpsimd.dma_start(out=y[mc * P:(mc + 1) * P, :], in_=ysb[:])

                if "noout1" not in ablate:
                    out_proj(O1_sb, wo1_sb, bo1_sb, y1)
                if "noout2" not in ablate:
                    out_proj(O2_sb, wo2_sb, bo2_sb, y2)

_NC_CACHE: bacc.Bacc | None = None


def _compile(nc: bacc.Bacc) -> None:
    """nc.compile() with the ACT-table pass pinned to one table set.

    All activation funcs used here (Exp, Identity, Copy) live in the
    'natural_log_exp_and_others' set. The default insert_act_table_loads pass
    picks the first set containing each func, which can alternate sets and
    insert a LoadActFuncSet before nearly every activation (each very
    expensive on hardware). Restricting every other set to empty (keeping
    dict order, so set ids stay valid) makes every func resolve to the one
    set -> a single load.
    """
    import concourse.bacc as _bacc_mod

    orig = _bacc_mod.get_activation_tables
    keep = "natural_log_exp_and_others"

    def pinned(arch):
        tables = orig(arch)
        assert keep in tables
        return {k: (v if k == keep else set()) for k, v in tables.items()}

    _bacc_mod.get_activation_tables = pinned
    try:
        nc.compile()
    finally:
        _bacc_mod.get_activation_tables = orig


def build_nc() -> bacc.Bacc:
    global _NC_CACHE
    if _NC_CACHE is None:
        nc = bacc.Bacc("TRN2", target_bir_lowering=False, debug=False)
        _emit(nc)
        _compile(nc)
        _NC_CACHE = nc
    return _NC_CACHE


def make_in_maps(inputs: dict[str, np.ndarray]) -> list[dict[str, np.ndarray]]:
    bf = ml_dtypes.bfloat16
    i1 = np.asarray(inputs["input1"], np.float32).reshape(B, C, S)
    i2 = np.asarray(inputs["input2"], np.float32).reshape(B, C, S)
    k1_w = np.asarray(inputs["k1_w"], np.float32)
    k2_w = np.asarray(inputs["k2_w"], np.float32)
    v1_w = np.asarray(inputs["v1_w"], np.float32)
    v2_w = np.asarray(inputs["v2_w"], np.float32)
    o1_w = np.asarray(inputs["o1_w"], np.float32)
    o2_w = np.asarray(inputs["o2_w"], np.float32)
    k1_b = np.asarray(inputs["k1_b"], np.float32)
    k2_b = np.asarray(inputs["k2_b"], np.float32)
    v1_b = np.asarray(inputs["v1_b"], np.float32)
    v2_b = np.asarray(inputs["v2_b"], np.float32)
    o1_b = np.asarray(inputs["o1_b"], np.float32)
    o2_b = np.asarray(inputs["o2_b"], np.float32)

    shared = {
        "wk1": np.ascontiguousarray(k1_w.T).astype(bf),
        "wk2": np.ascontiguousarray(k2_w.T).astype(bf),
        "wv1": np.ascontiguousarray(v1_w.T).astype(bf),
        "wv2": np.ascontiguousarray(v2_w.T).astype(bf),
        "wo1": np.ascontiguousarray(o1_w.T).astype(bf),
        "wo2": np.ascontiguousarray(o2_w.T).astype(bf),
        "bk1": np.ascontiguousarray(k1_b.reshape(J // P, P).T),
        "bk2": np.ascontiguousarray(k2_b.reshape(J // P, P).T),
        # V-bias folds into the output-projection bias:
        #   out1 uses v2  ->  bo1_eff = o1_b + o1_w @ v2_b
        "bo1": np.ascontiguousarray((o1_b + o1_w @ v2_b).reshape(NCC, P).T),
        "bo2": np.ascontiguousarray((o2_b + o2_w @ v1_b).reshape(NCC, P).T),
    }
    return [
        {"x1": i1[b].astype(bf), "x2": i2[b].astype(bf), **shared}
        for b in range(B)
    ]


def kernel(**inputs) -> tuple[np.ndarray, np.ndarray]:
    from concourse.bass_utils import run_bass_kernel_spmd

    nc = build_nc()
    in_maps = make_in_maps(inputs)
    for _attempt in range(3):
        res = run_bass_kernel_spmd(nc, in_maps, list(range(B))).results
        out1 = np.stack([res[b]["y1"] for b in range(B)]).reshape(B, C, 32, 32)
        out2 = np.stack([res[b]["y2"] for b in range(B)]).reshape(B, C, 32, 32)
        # very rarely the first execution on a cold device returns NaNs;
        # re-running the same NEFF has always produced clean output
        if not (np.isnan(out1).any() or np.isnan(out2).any()):
            break
    return out1.astype(np.float32), out2.astype(np.float32)


# revision 31
# speedup vs baseline: 1.0388x; 1.0388x over previous
"""Bi-directional cross-attention kernel for Trainium2 (8 NeuronCores).

Sharding: data-parallel over batch B=8 -> one batch element per core (SPMD,
no collectives). Each core computes the full bidirectional cross-attention
for its batch element.

Per-core layout strategy (C=256 channels, S=1024 tokens, 8 heads x 64 dim):
  - K1col/K2col: [512, 1024]  (head-major rows on partitions, tokens free)
  - V1aug/V2aug: row layout [1024 tokens, 8*(64+1)] with a ones-column per
    head; the ones-column makes the attention matmul emit the softmax
    denominator as PSUM row 64 for free.
  - scores are never max-shifted (values ~N(0, 0.01) after the 1/8 scale,
    exp is safe); exp(0.125*s) fused into the PSUM->SBUF copy on ScalarE.
  - E^T produced by xbar DMA transposes of the E tiles (issued right after
    each exp, all on the SP HWDGE ring -- mixing rings corrupts data on this
    runtime) -- keeps the second softmax orientation OFF the ScalarE
    critical path entirely; the transfers hide under compute.
  - normalization: the PSUM accumulator is staged to SBUF with one copy so
    its bank frees immediately; 1/denom via the single-instruction
    custom-DVE reciprocal_approx_fast (~51 ULP, plenty for a softmax
    denominator; it must read a base-0 full tile -- partition-offset APs
    are mishandled on HW), broadcast across partitions via a K=1 matmul
    against an all-ones row, then one VectorE multiply (in1 straight from
    PSUM).
  - output projection packs head PAIRS (K=128 contraction instead of two
    K=64 chains), halving its PE time.
  - output projection biases (including the folded V-bias contribution) are
    applied per-partition during the final PSUM->SBUF copy on VectorE.
  - iters>1 wraps the body in a tc.For_i hardware loop (body emitted once,
    replayed on device) -- used by test.py to measure marginal per-iteration
    HW time without NEFF-size-dependent host overhead.
"""

import os
import sys

for _p in ("/opt/trn_rl_repo", os.path.expanduser("~/.axon_site/_ro/trn_rl_repo")):
    if os.path.isdir(_p) and _p not in sys.path:
        sys.path.insert(0, _p)

import numpy as np
import ml_dtypes

import concourse.bass as bass
import concourse.tile as tile
import concourse.mybir as mybir
from concourse import bacc

BF16 = mybir.dt.bfloat16
F32 = mybir.dt.float32
AF = mybir.ActivationFunctionType
ALU = mybir.AluOpType

B = 8
C = 256          # channels per image
S = 1024         # tokens per image (32*32)
NH = 8           # heads
HD = 64          # head dim
J = NH * HD      # 512
P = 128
NCC = C // P     # 2 channel chunks
NQ = S // P      # 8 token chunks
NKB = S // 512   # 2 psum banks across tokens
HB = HD + 1      # head block width in V-aug (64 d + ones col)
NP = NH // 2     # head pairs


def _emit(nc: bass.Bass, iters: int = 1,
          transpose_mode: str = "xbar_sp",
          ones_engine: str = "pool",
          norm_mode: str = "recip",
          unroll: int = 1,
          ablate: frozenset = frozenset()) -> None:
    x1 = nc.declare_dram_parameter("x1", [C, S], BF16, isOutput=False)
    x2 = nc.declare_dram_parameter("x2", [C, S], BF16, isOutput=False)
    wk1 = nc.declare_dram_parameter("wk1", [C, J], BF16, isOutput=False)
    wk2 = nc.declare_dram_parameter("wk2", [C, J], BF16, isOutput=False)
    wv1 = nc.declare_dram_parameter("wv1", [C, J], BF16, isOutput=False)
    wv2 = nc.declare_dram_parameter("wv2", [C, J], BF16, isOutput=False)
    wo1 = nc.declare_dram_parameter("wo1", [J, C], BF16, isOutput=False)
    wo2 = nc.declare_dram_parameter("wo2", [J, C], BF16, isOutput=False)
    bk1 = nc.declare_dram_parameter("bk1", [P, J // P], F32, isOutput=False)
    bk2 = nc.declare_dram_parameter("bk2", [P, J // P], F32, isOutput=False)
    bo1 = nc.declare_dram_parameter("bo1", [P, NCC], F32, isOutput=False)
    bo2 = nc.declare_dram_parameter("bo2", [P, NCC], F32, isOutput=False)
    y1 = nc.declare_dram_parameter("y1", [C, S], F32, isOutput=True)
    y2 = nc.declare_dram_parameter("y2", [C, S], F32, isOutput=True)

    with tile.TileContext(nc) as tc:
        with (
            tc.tile_pool(name="const", bufs=1) as cp,
            tc.tile_pool(name="work", bufs=2) as wp,
            tc.tile_pool(name="norm", bufs=4) as np_,
            tc.tile_pool(name="psA", bufs=2, space="PSUM") as psA,
            tc.tile_pool(name="psB", bufs=2, space="PSUM") as psB,
            tc.tile_pool(name="psC", bufs=2, space="PSUM") as psC,
        ):
            # all-ones row at partition 64, used to broadcast the recip row
            # across partitions via a K=1 matmul.
            ones_t = cp.tile([HD + 1, HD], BF16, tag="ones_t", name="ones_t")
            nc.vector.memset(ones_t[:], 1.0)

            def _body_loop():
                if iters == 1:
                    yield 0
                else:
                    # hardware loop: one body copy in the NEFF, replayed
                    # iters times back-to-back on device. unroll>1 emits the
                    # body several times per loop trip (fewer back-edge
                    # all-engine barriers).
                    assert iters % unroll == 0
                    with tc.For_i(0, iters // unroll, 1):
                        for _u in range(unroll):
                            yield _u

            def load(dram, shape, dtype, tag):
                # SWDGE (Pool-issued) ring: keeps the SP HWDGE ring free
                # for the E^T xbar transposes
                t = cp.tile(shape, dtype, tag=tag, name=tag)
                nc.gpsimd.dma_start(out=t[:], in_=dram[:])
                return t

            # weights and biases are loop-invariant: loaded once, resident in
            # SBUF across iterations (weight-stationary steady state)
            wk1_sb = [load(wk1[cc * P:(cc + 1) * P, :], [P, J], BF16, f"wk1_{cc}")
                      for cc in range(NCC)]
            wk2_sb = [load(wk2[cc * P:(cc + 1) * P, :], [P, J], BF16, f"wk2_{cc}")
                      for cc in range(NCC)]
            wv1_sb = [load(wv1[cc * P:(cc + 1) * P, :], [P, J], BF16, f"wv1_{cc}")
                      for cc in range(NCC)]
            wv2_sb = [load(wv2[cc * P:(cc + 1) * P, :], [P, J], BF16, f"wv2_{cc}")
                      for cc in range(NCC)]
            # o-projection weights: one [128, C] tile per head PAIR so the
            # out-projection contracts K=128 per matmul.
            wo1_sb = [load(wo1[hp * P:(hp + 1) * P, :], [P, C], BF16, f"wo1_{hp}")
                      for hp in range(NP)]
            wo2_sb = [load(wo2[hp * P:(hp + 1) * P, :], [P, C], BF16, f"wo2_{hp}")
                      for hp in range(NP)]
            bk1_sb = load(bk1, [P, J // P], F32, "bk1")
            bk2_sb = load(bk2, [P, J // P], F32, "bk2")
            bo1_sb = load(bo1, [P, NCC], F32, "bo1")
            bo2_sb = load(bo2, [P, NCC], F32, "bo2")

            for _it in _body_loop():
                # ---- load activations -------------------------------------------
                def load2(dram, shape, dtype, tag):
                    # bufs=2 so the next loop copy's loads overlap this
                    # copy's compute tail
                    t = cp.tile(shape, dtype, tag=tag, name=tag, bufs=2)
                    nc.gpsimd.dma_start(out=t[:], in_=dram[:])
                    return t

                x1_sb = [load2(x1[cc * P:(cc + 1) * P, :], [P, S], BF16, f"x1_{cc}")
                         for cc in range(NCC)]
                x2_sb = [load2(x2[cc * P:(cc + 1) * P, :], [P, S], BF16, f"x2_{cc}")
                         for cc in range(NCC)]

                # ---- K projections: Kcol[j, s] = sum_c wk[c, j] * x[c, s] + bk ---
                def k_proj(x_sb, w_sb, b_sb, tag):
                    out = []
                    for m in range(J // P):
                        ps = psA.tile([P, S], F32, tag="pe", name="pe")
                        for nb in range(NKB):
                            for cc in range(NCC):
                                nc.tensor.matmul(
                                    ps[:, nb * 512:(nb + 1) * 512],
                                    lhsT=w_sb[cc][:, m * P:(m + 1) * P],
                                    rhs=x_sb[cc][:, nb * 512:(nb + 1) * 512],
                                    start=(cc == 0), stop=(cc == NCC - 1),
                                )
                        k_sb = cp.tile([P, S], BF16, tag=f"{tag}_{m}", name=f"{tag}_{m}")
                        nc.vector.tensor_scalar(k_sb[:], ps[:],
                                                b_sb[:, m:m + 1], None, ALU.add)
                        out.append(k_sb)
                    return out

                K1_sb = k_proj(x1_sb, wk1_sb, bk1_sb, "k1")
                K2_sb = k_proj(x2_sb, wk2_sb, bk2_sb, "k2")

                # ---- V projections into augmented row layout ---------------------
                # Vaug[qc] : [128 tokens, 8*(64+1)] ; per-head 64 values + ones col
                def v_proj(x_sb, w_sb, tag):
                    out = []
                    for qc in range(NQ):
                        ps = psB.tile([P, J], F32, tag="po", name="po")
                        for cc in range(NCC):
                            nc.tensor.matmul(
                                ps[:],
                                lhsT=x_sb[cc][:, qc * P:(qc + 1) * P],
                                rhs=w_sb[cc][:],
                                start=(cc == 0), stop=(cc == NCC - 1),
                            )
                        va = cp.tile([P, NH * HB], BF16, tag=f"{tag}_{qc}", name=f"{tag}_{qc}")
                        va_v = va[:].rearrange("p (h c) -> p h c", c=HB)
                        ps_v = ps[:].rearrange("p (h c) -> p h c", c=HD)
                        nc.vector.tensor_copy(va_v[:, :, 0:HD], ps_v)
                        if ones_engine == "pool":
                            nc.gpsimd.memset(va_v[:, :, HD:HB], 1.0)
                        else:
                            nc.vector.memset(va_v[:, :, HD:HB], 1.0)
                        out.append(va)
                    return out

                V1a_sb = v_proj(x1_sb, wv1_sb, "v1a")
                V2a_sb = v_proj(x2_sb, wv2_sb, "v2a")

                # output tiles per head PAIR: [128, S] (head A rows 0-63,
                # head B rows 64-127) so out_proj contracts K=128.
                O1_sb = [cp.tile([P, S], BF16, tag=f"o1_{hp}", name=f"o1_{hp}")
                         for hp in range(NP)]
                O2_sb = [cp.tile([P, S], BF16, tag=f"o2_{hp}", name=f"o2_{hp}")
                         for hp in range(NP)]

                def normalize(po, o_sb, r0, nb):
                    if "nonorm" in ablate:  # timing ablation only
                        nc.vector.tensor_copy(
                            o_sb[r0:r0 + HD, nb * 512:(nb + 1) * 512], po[0:HD, :])
                        return
                    """po: [65, 512] psum (rows 0..63 unnormalized out, row 64
                    the softmax denominator). Writes o_sb[r0:r0+64, nb*512:...].
                    One DVE copy stages po to SBUF so the PSUM slot frees
                    immediately (the accumulation chains never wait on the
                    normalize latency); then 1/denom on DVE (single custom
                    op over the full base-0 tile -- the op mishandles
                    partition-offset APs on HW), broadcast across the 64
                    partitions via a K=1 matmul, one DVE mult (in1 read
                    straight from PSUM)."""
                    u = np_.tile([HD + 1, 512], F32, tag="u", name="u")
                    nc.vector.tensor_copy(u[:], po[:])
                    rb = np_.tile([HD + 1, 512], BF16, tag="rb", name="rb")
                    if norm_mode == "recip":
                        # recip runs straight off PSUM (base-0 full tile);
                        # the bf16 u staging copy runs in parallel
                        rr = np_.tile([HD + 1, 512], F32, tag="rr", name="rr")
                        nc.vector.reciprocal_approx_fast(out=rr[:], in_=po[:])
                        nc.gpsimd.tensor_copy(rb[HD:HD + 1, :], rr[HD:HD + 1, :])
                    else:  # ln/exp on ScalarE (baseline scheme)
                        lt = np_.tile([HD + 1, 512], F32, tag="lt", name="lt")
                        nc.scalar.activation(lt[HD:HD + 1, :], po[HD:HD + 1, :], AF.Ln)
                        nc.scalar.activation(rb[HD:HD + 1, :], lt[HD:HD + 1, :], AF.Exp,
                                             scale=-1.0)
                    bc_ps = psC.tile([HD, 512], F32, tag="bc", name="bc_ps")
                    nc.tensor.matmul(bc_ps[:], lhsT=ones_t[HD:HD + 1, :],
                                     rhs=rb[HD:HD + 1, :], start=True, stop=True)
                    nc.vector.tensor_tensor(
                        out=o_sb[r0:r0 + HD, nb * 512:(nb + 1) * 512],
                        in0=u[0:HD, :], in1=bc_ps[:], op=ALU.mult)

                if "floor" in ablate:  # timing ablation: no attention at all
                    for hp0 in range(NP):
                        nc.vector.memset(O1_sb[hp0][:], 0.001)
                        nc.vector.memset(O2_sb[hp0][:], 0.001)
                # ---- attention, processed in head pairs -------------------------
                # the two heads of a pair occupy array row groups 0-63 and
                # 64-127 (lhsT/rhs partition base auto-derives tile_position),
                # so their K=64 score matmuls run concurrently on the PE.
                for hp in (range(0) if "floor" in ablate else range(NP)):
                    pair = (2 * hp, 2 * hp + 1)
                    e_sb = {h: [] for h in pair}
                    et = {}
                    for h in pair:
                        et[h] = wp.tile([P, NQ, S], BF16, tag=f"et{h % 2}",
                                        name="et", bufs=1)
                    for qc in range(NQ):
                        pse = {}
                        for h in pair:
                            r0 = HD * (h % 2)
                            ps = psA.tile([P, S], F32, tag="pe", name="pe")
                            for nb in range(NKB):
                                nc.tensor.matmul(
                                    ps[:, nb * 512:(nb + 1) * 512],
                                    lhsT=K1_sb[hp][r0:r0 + HD, qc * P:(qc + 1) * P],
                                    rhs=K2_sb[hp][r0:r0 + HD, nb * 512:(nb + 1) * 512],
                                    start=True, stop=True,
                                )
                            pse[h] = ps
                        for h in pair:
                            e = wp.tile([P, S], BF16, tag=f"e{qc}", name=f"e{qc}",
                                        bufs=4)
                            if "smallexp" in ablate:  # timing ablation only
                                nc.scalar.activation(e[:, 0:P], pse[h][:, 0:P],
                                                     AF.Exp, scale=0.125)
                            else:
                                nc.scalar.activation(e[:], pse[h][:], AF.Exp,
                                                     scale=0.125)
                            e_sb[h].append(e)
                            if (transpose_mode in ("xbar", "xbar_sp", "xbar_h")
                                    and "noet" not in ablate):
                                # E^T tile for this q-chunk via xbar DMA
                                # transpose; "xbar" alternates HWDGE rings
                                # (SP / ACT) per chunk, "xbar_sp" pins all to
                                # SP, "xbar_h" pins per-head (each et tile
                                # written by exactly one ring)
                                if transpose_mode == "xbar_sp":
                                    eng = nc.sync
                                elif transpose_mode == "xbar_h":
                                    eng = nc.sync if h % 2 == 0 else nc.scalar
                                else:
                                    eng = nc.sync if (qc + h) % 2 == 0 else nc.scalar
                                eng.dma_start(
                                    out=et[h][:, :, qc * P:(qc + 1) * P],
                                    in_=e[:],
                                    transpose=True,
                                )

                    if transpose_mode == "recompute":
                        # E^T = exp(0.125 * K2_h^T K1_h) computed directly
                        for h in pair:
                            r0 = HD * (h % 2)
                            for kc in range(NQ):
                                ps = psA.tile([P, S], F32, tag="pe", name="pe")
                                for nb in range(NKB):
                                    nc.tensor.matmul(
                                        ps[:, nb * 512:(nb + 1) * 512],
                                        lhsT=K2_sb[hp][r0:r0 + HD,
                                                       kc * P:(kc + 1) * P],
                                        rhs=K1_sb[hp][r0:r0 + HD,
                                                      nb * 512:(nb + 1) * 512],
                                        start=True, stop=True,
                                    )
                                nc.scalar.activation(et[h][:, kc, :], ps[:],
                                                     AF.Exp, scale=0.125)

                    # out2 chains (need only E) for both heads first, giving
                    # the xbar transposes time to land before out1 needs E^T
                    for h in pair:
                        if "noout2" in ablate:
                            break
                        r0 = HD * (h % 2)
                        # out2[d, k] = sum_q V1[q, h*64+d]*E[q, k] (+denominator)
                        for nb in range(NKB):
                            po = psB.tile([HB, 512], F32, tag="po", name="po")
                            for qc in range(NQ):
                                nc.tensor.matmul(
                                    po[:],
                                    lhsT=V1a_sb[qc][:, h * HB:(h + 1) * HB],
                                    rhs=e_sb[h][qc][:, nb * 512:(nb + 1) * 512],
                                    start=(qc == 0), stop=(qc == NQ - 1),
                                )
                            normalize(po, O2_sb[hp], r0, nb)

                    for h in pair:
                        if "noout1" in ablate:
                            break
                        r0 = HD * (h % 2)
                        # out1[d, q] = sum_k V2[k, h*64+d]*E[q, k] (+denominator)
                        for nb in range(NKB):
                            po = psB.tile([HB, 512], F32, tag="po", name="po")
                            for kc in range(NQ):
                                rhs = (e_sb[h][kc][:, nb * 512:(nb + 1) * 512]
                                       if "noet" in ablate else
                                       et[h][:, kc, nb * 512:(nb + 1) * 512])
                                nc.tensor.matmul(
                                    po[:],
                                    lhsT=V2a_sb[kc][:, h * HB:(h + 1) * HB],
                                    rhs=rhs,
                                    start=(kc == 0), stop=(kc == NQ - 1),
                                )
                            normalize(po, O1_sb[hp], r0, nb)

                # ---- output projections (head-pair K=128 contraction) -----------
                def out_proj(o_sb, wo_sb, bo_sb, y):
                    for mc in range(NCC):
                        ps = psA.tile([P, S], F32, tag="pe", name="pe")
                        for nb in range(NKB):
                            for hp in range(NP):
                                nc.tensor.matmul(
                                    ps[:, nb * 512:(nb + 1) * 512],
                                    lhsT=wo_sb[hp][:, mc * P:(mc + 1) * P],
                                    rhs=o_sb[hp][:, nb * 512:(nb + 1) * 512],
                                    start=(hp == 0), stop=(hp == NP - 1),
                                )
                        ysb = wp.tile([P, S], F32, tag="y", name="y")
                        nc.vector.tensor_scalar(ysb[:], ps[:],
                                                bo_sb[:, mc:mc + 1], None, ALU.add)
                        nc.gpsimd.dma_start(out=y[mc * P:(mc + 1) * P, :], in_=ysb[:])

                if "noout1" not in ablate:
                    out_proj(O1_sb, wo1_sb, bo1_sb, y1)
                if "noout2" not in ablate:
                    out_proj(O2_sb, wo2_sb, bo2_sb, y2)

_NC_CACHE: bacc.Bacc | None = None


def _compile(nc: bacc.Bacc) -> None:
    """nc.compile() with the ACT-table pass pinned to one table set.

    All activation funcs used here (Exp, Identity, Copy) live in the
    'natural_log_exp_and_others' set. The default insert_act_table_loads pass
    picks the first set containing each func, which can alternate sets and
    insert a LoadActFuncSet before nearly every activation (each very
    expensive on hardware). Restricting every other set to empty (keeping
    dict order, so set ids stay valid) makes every func resolve to the one
    set -> a single load.
    """
    import concourse.bacc as _bacc_mod

    orig = _bacc_mod.get_activation_tables
    keep = "natural_log_exp_and_others"

    def pinned(arch):
        tables = orig(arch)
        assert keep in tables
        return {k: (v if k == keep else set()) for k, v in tables.items()}

    _bacc_mod.get_activation_tables = pinned
    try:
        nc.compile()
    finally:
        _bacc_mod.get_activation_tables = orig


def build_nc() -> bacc.Bacc:
    global _NC_CACHE
    if _NC_CACHE is None:
        nc = bacc.Bacc("TRN2", target_bir_lowering=False, debug=False)
        _emit(nc)
        _compile(nc)
        _NC_CACHE = nc
    return _NC_CACHE


def make_in_maps(inputs: dict[str, np.ndarray]) -> list[dict[str, np.ndarray]]:
    bf = ml_dtypes.bfloat16
    i1 = np.asarray(inputs["input1"], np.float32).reshape(B, C, S)
    i2 = np.asarray(inputs["input2"], np.float32).reshape(B, C, S)
    k1_w = np.asarray(inputs["k1_w"], np.float32)
    k2_w = np.asarray(inputs["k2_w"], np.float32)
    v1_w = np.asarray(inputs["v1_w"], np.float32)
    v2_w = np.asarray(inputs["v2_w"], np.float32)
    o1_w = np.asarray(inputs["o1_w"], np.float32)
    o2_w = np.asarray(inputs["o2_w"], np.float32)
    k1_b = np.asarray(inputs["k1_b"], np.float32)
    k2_b = np.asarray(inputs["k2_b"], np.float32)
    v1_b = np.asarray(inputs["v1_b"], np.float32)
    v2_b = np.asarray(inputs["v2_b"], np.float32)
    o1_b = np.asarray(inputs["o1_b"], np.float32)
    o2_b = np.asarray(inputs["o2_b"], np.float32)

    shared = {
        "wk1": np.ascontiguousarray(k1_w.T).astype(bf),
        "wk2": np.ascontiguousarray(k2_w.T).astype(bf),
        "wv1": np.ascontiguousarray(v1_w.T).astype(bf),
        "wv2": np.ascontiguousarray(v2_w.T).astype(bf),
        "wo1": np.ascontiguousarray(o1_w.T).astype(bf),
        "wo2": np.ascontiguousarray(o2_w.T).astype(bf),
        "bk1": np.ascontiguousarray(k1_b.reshape(J // P, P).T),
        "bk2": np.ascontiguousarray(k2_b.reshape(J // P, P).T),
        # V-bias folds into the output-projection bias:
        #   out1 uses v2  ->  bo1_eff = o1_b + o1_w @ v2_b
        "bo1": np.ascontiguousarray((o1_b + o1_w @ v2_b).reshape(NCC, P).T),
        "bo2": np.ascontiguousarray((o2_b + o2_w @ v1_b).reshape(NCC, P).T),
    }
    return [
        {"x1": i1[b].astype(bf), "x2": i2[b].astype(bf), **shared}
        for b in range(B)
    ]


def kernel(**inputs) -> tuple[np.ndarray, np.ndarray]:
    from concourse.bass_utils import run_bass_kernel_spmd

    nc = build_nc()
    in_maps = make_in_maps(inputs)
    for _attempt in range(3):
        res = run_bass_kernel_spmd(nc, in_maps, list(range(B))).results
        out1 = np.stack([res[b]["y1"] for b in range(B)]).reshape(B, C, 32, 32)
        out2 = np.stack([res[b]["y2"] for b in range(B)]).reshape(B, C, 32, 32)
        # very rarely the first execution on a cold device returns NaNs;
        # re-running the same NEFF has always produced clean output
        if not (np.isnan(out1).any() or np.isnan(out2).any()):
            break
    return out1.astype(np.float32), out2.astype(np.float32)
